# revision 17
# baseline (speedup 1.0000x reference)
"""AdaConv2d (per-pixel 3x3 dynamic conv) on 8 TRN2 NeuronCores.

out[b,c,h,w] = sum_t x_pad[b,c,h+dh(t),w+dw(t)] * dk[b,c,t,h,w]

Sharding: pure data parallel over batch (B=8 -> one batch element per core).

Per-core layout: partition p = 2c+s (c = channel 0..63, s = H-half 0..1); each
partition holds its half-plane of x as a 66-row bf16 SBUF tile (row 0 / row 65
are parity-masked halo rows, so every tap's product is one uniform full-height
DVE mul). dynamic_kernel streams through on the sync-engine DMA ring in
(row-block, tap) f32 tiles; the ACT engine converts each tile to bf16
(~3.4us/tile, ~65us total - under the ~102us DMA stream) so the DVE tap
products run in all-bf16 2x perf mode (~2.2us per 32-row tap, ~40us total).
DVE therefore can never pace the endgame: the kernel end is stream-end plus a
short convert->mul->matmul->copy->store chain on the final 8-row slice.
Row blocks: rows 0..32 as nine 32-row taps (16KB/lane packets, the DMA sweet
spot), rows 32..64 as a merged group accumulated in 16/8/8-row sub-blocks
whose closing taps are loaded as row-slices so each sub-block drains while
later slices still stream. W-boundary tap columns are memset (on gpsimd,
whose queue carries nothing else) in the bf16 product tile; the 9-tap sum
accumulates on the TensorEngine via identity-matmul into PSUM; ACT drains
PSUM to bf16 and issues the stores. Memory-bound: ~44 MB/core HBM traffic at
~400 GB/s/core effective.
"""

import numpy as np

from concourse import bacc, bass, tile
from concourse import mybir
from concourse.ap import AP
from concourse.bass_utils import run_bass_kernel_spmd
from concourse.masks import make_identity

F32 = mybir.dt.float32
BF16 = mybir.dt.bfloat16

B, C, H, W = 8, 64, 128, 128
K = 3
NTAP = K * K
NCORES = 8

HALF = H // 2           # 64 rows per half-plane
XROWS = HALF + 2        # 66 padded rows per partition

_CACHED_NC = None


def _emit(tc, nc, x_ap, dk_ap, pm_dram, out_ap):
    ctx_pools = []

    def pool(name, bufs, space=bass.MemorySpace.SBUF):
        p = tc.tile_pool(name=name, bufs=bufs, space=space)
        ctx_pools.append(p)
        return p.__enter__()

    try:
        const_pool = pool("const", 1)
        xbf_pool = pool("xbf", 1)
        dk_pool = pool("dk", 6)
        tmp_pool = pool("tmp", 7)
        out_pool = pool("osb", 3)
        psum_pool = pool("ps", 8, space=bass.MemorySpace.PSUM)

        identity = const_pool.tile([128, 128], BF16, name="identity")
        make_identity(nc, identity)

        # Partition p = 2c+s (c = channel, s = H-half). x_bf row 1+f holds
        # x[c, 64s+f]; row 0 is the top halo (x row 63 of the other half,
        # nonzero only for odd p) and row 65 the bottom halo (x row 64,
        # nonzero only for even p): every tap's product is ONE uniform
        # full-height DVE mul. W-boundary tap columns are memset in tmp.
        pm_ap = const_pool.tile([128, 2], F32, name="pm_ap")
        nc.scalar.dma_start(out=pm_ap[:], in_=pm_dram)
        halo_raw = const_pool.tile([128, 2, W], F32, name="halo_raw")
        nc.scalar.dma_start(
            out=halo_raw[:, 0:1, :],
            in_=AP(x_ap.tensor, HALF * W, [[H * W, C], [W, 2], [1, W]]),
        )
        nc.scalar.dma_start(
            out=halo_raw[:, 1:2, :],
            in_=AP(x_ap.tensor, (HALF - 2) * W, [[H * W, C], [W, 2], [1, W]]),
        )
        x_bf = xbf_pool.tile([128, XROWS, W], BF16, name="x_bf")
        nc.vector.tensor_scalar_mul(
            x_bf[:, 0:1, :], halo_raw[:, 1:2, :], pm_ap[:, 1:2]
        )
        nc.vector.tensor_scalar_mul(
            x_bf[:, HALF + 1 : HALF + 2, :], halo_raw[:, 0:1, :], pm_ap[:, 0:1]
        )

        def load_x_stage(lo, hi):
            # f32 staging load on the sync ring, ACT converts into x_bf
            xs = dk_pool.tile([128, hi - lo, W], F32, name=f"xs{lo}", tag="dk")
            nc.sync.dma_start(
                out=xs[:],
                in_=AP(
                    x_ap.tensor,
                    lo * W,
                    [[H * W, C], [HALF * W, 2], [W, hi - lo], [1, W]],
                ),
            )
            nc.scalar.copy(out=x_bf[:, lo + 1 : hi + 1, :], in_=xs[:])

        def load_dk(t, rlo, rhi):
            # f32 tile streams in on the sync ring; ACT converts it straight
            # into the bf16 product tile (tmp), which the DVE then multiplies
            # by x IN PLACE (all-bf16 operands -> DVE 2x perf mode).
            dk_t = dk_pool.tile([128, rhi - rlo, W], F32, name="dk_t", tag="dk")
            nc.sync.dma_start(
                out=dk_t[:],
                in_=AP(
                    dk_ap.tensor,
                    t * H * W + rlo * W,
                    [[NTAP * H * W, C], [HALF * W, 2], [W, rhi - rlo], [1, W]],
                ),
            )
            tmp = tmp_pool.tile([128, rhi - rlo, W], BF16, name="tmp", tag="tmp")
            nc.scalar.copy(out=tmp[:], in_=dk_t[:])
            return tmp

        def col_slices(dw):
            # valid output columns for this tap; the one boundary column
            # (reading x col -1 or W) contributes zero
            wo = slice(1, W) if dw < 0 else slice(0, W - 1) if dw > 0 else slice(0, W)
            return wo, slice(wo.start + dw, wo.stop + dw)

        # ---- block 0: output rows 0..32, nine monolithic 32-row taps
        # (16KB/lane packets). Its drain happens mid-stream with slack.
        load_x_stage(0, 32)
        load_x_stage(32, HALF)
        ps0 = [
            psum_pool.tile([128, 4, 128], F32, name=f"ps0_{j}", tag="ps")
            for j in range(8)
        ]
        order0 = [3, 4, 5, 6, 7, 0, 1, 2]
        for ti, t in enumerate(order0):
            dh, dw = t // K - 1, t % K - 1
            tmp = load_dk(t, 0, 32)
            wo, wx = col_slices(dw)
            if dw != 0:
                zc = slice(0, 1) if dw < 0 else slice(W - 1, W)
                nc.gpsimd.memset(tmp[:, :, zc], 0.0)
            nc.vector.tensor_mul(
                tmp[:, :, wo], x_bf[:, dh + 1 : dh + 33, wx], tmp[:, :, wo]
            )
            for j in range(8):
                nc.tensor.matmul(
                    ps0[j][:],
                    identity[:],
                    tmp[:, 4 * j : 4 * j + 4, :],
                    start=(ti == 0),
                    stop=False,
                )
        # closing tap in two 16-row halves: tiles 0-3 stop (and drain via
        # GpSimd) earlier, so the group's psum-slot handoff stalls less
        out_sb0 = out_pool.tile([128, 32, W], BF16, name="out_sb0", tag="osb")
        for h in range(2):
            dh, dw = 8 // K - 1, 8 % K - 1
            tmp = load_dk(8, 16 * h, 16 * h + 16)
            wo, wx = col_slices(dw)
            zc = slice(W - 1, W)
            nc.gpsimd.memset(tmp[:, :, zc], 0.0)
            nc.vector.tensor_mul(
                tmp[:, :, wo],
                x_bf[:, 16 * h + dh + 1 : 16 * h + dh + 17, wx],
                tmp[:, :, wo],
            )
            for j in range(4):
                nc.tensor.matmul(
                    ps0[4 * h + j][:],
                    identity[:],
                    tmp[:, 4 * j : 4 * j + 4, :],
                    start=False,
                    stop=True,
                )
            # drain this half's psum tiles immediately so the group's
            # matmuls can recycle them as early as possible
            for j in range(4):
                nc.vector.tensor_copy(
                    out_sb0[:, 16 * h + 4 * j : 16 * h + 4 * j + 4, :],
                    ps0[4 * h + j][:],
                )
        nc.scalar.dma_start(
            out=AP(out_ap.tensor, 0, [[H * W, C], [HALF * W, 2], [W, 32], [1, W]]),
            in_=out_sb0[:],
        )

        # ---- group: output rows 32..64 as three sub-blocks (16/8/8 rows)
        # sharing mostly-32-row dk loads (16KB packets); the closing tap
        # (t=4, center: no memset) is loaded in three row-slices so the
        # post-stream drain is a single 8-row mul -> 2 matmuls -> copies
        # -> store.
        G = 32
        sbs = []  # (group-row lo, hi, psum tiles)
        for lo, hi in [(0, 16), (16, 28), (28, 32)]:
            tiles = [
                psum_pool.tile([128, 4, 128], F32, name=f"psg_{lo}_{j}", tag="ps")
                for j in range((hi - lo) // 4)
            ]
            sbs.append((lo, hi, tiles))
        deferred_stores = []

        def group_step(t, glo, ghi, tmp):
            dh, dw = t // K - 1, t % K - 1
            wo, wx = col_slices(dw)
            # ONE fused mul per loaded slice (the sub-block split only
            # matters for psum/matmul grouping)
            if dw != 0:
                zc = slice(0, 1) if dw < 0 else slice(W - 1, W)
                nc.gpsimd.memset(tmp[:, :, zc], 0.0)
            nc.vector.tensor_mul(
                tmp[:, :, wo],
                x_bf[:, G + glo + dh + 1 : G + ghi + dh + 1, wx],
                tmp[:, :, wo],
            )
            for lo, hi, tiles in sbs:
                a, b_ = max(lo, glo), min(hi, ghi)
                if a >= b_:
                    continue
                for j in range(len(tiles)):
                    tr = lo + 4 * j  # group row of this psum tile
                    if tr < a or tr >= b_:
                        continue
                    nc.tensor.matmul(
                        tiles[j][:],
                        identity[:],
                        tmp[:, tr - glo : tr - glo + 4, :],
                        start=(t == 5),
                        stop=(t == 4),
                    )
                if t == 4:
                    # this sub-block just closed: drain (B1/B2 on ACT, the
                    # final 4-row B3 on Vector, which is idle after its last
                    # mul) + store. B1/B2 store dispatch rides the sync ring
                    # (emitted after the final dk load, so it cannot block
                    # the stream).
                    out_sb = out_pool.tile(
                        [128, hi - lo, W], BF16, name=f"out_g{lo}", tag="osb"
                    )
                    for j in range(len(tiles)):
                        if lo == 28:
                            nc.vector.tensor_copy(
                                out_sb[:, 4 * j : 4 * j + 4, :], tiles[j][:]
                            )
                        else:
                            nc.scalar.copy(
                                out=out_sb[:, 4 * j : 4 * j + 4, :], in_=tiles[j][:]
                            )
                    out_dst = AP(
                        out_ap.tensor,
                        (G + lo) * W,
                        [[H * W, C], [HALF * W, 2], [W, hi - lo], [1, W]],
                    )
                    if lo in (0, 16):
                        deferred_stores.append((out_dst, out_sb))
                    else:
                        nc.scalar.dma_start(out=out_dst, in_=out_sb[:])

        # non-closing t3 slices ride mid-stream so the post-stream tail only
        # owes the three closing t4 slices' converts (2 MiB, ~3us of ACT)
        seq_main = [(5, 0, 32), (6, 0, 32), (7, 0, 32), (8, 0, 32), (0, 0, 32),
                    (3, 0, 16), (1, 0, 32), (3, 16, 32), (2, 0, 32)]
        for t, glo, ghi in seq_main:
            group_step(t, glo, ghi, load_dk(t, G + glo, G + ghi))
        # tail: emit ALL remaining loads + ACT converts first, so no drain
        # copy ever sits ahead of a convert in the ACT queue (head-of-line
        # blocking there delays the final muls past stream end)
        tail = [(4, 0, 16), (4, 16, 28), (4, 28, 32)]
        tail_tmps = [load_dk(t, G + glo, G + ghi) for t, glo, ghi in tail]
        for (t, glo, ghi), tmp in zip(tail, tail_tmps):
            group_step(t, glo, ghi, tmp)
        for dst, sb in deferred_stores:
            nc.sync.dma_start(out=dst, in_=sb[:])
    finally:
        for p in reversed(ctx_pools):
            p.__exit__(None, None, None)


def build_nc():
    global _CACHED_NC
    if _CACHED_NC is not None:
        return _CACHED_NC
    nc = bacc.Bacc("TRN2", target_bir_lowering=False, debug=False, num_devices=NCORES)
    x_ap = nc.dram_tensor("x", [C, H, W], F32, kind="ExternalInput").ap()
    dk_ap = nc.dram_tensor(
        "dynamic_kernel", [C, NTAP, H, W], F32, kind="ExternalInput"
    ).ap()
    pm_dram = nc.dram_tensor("pmask", [128, 2], F32, kind="ExternalInput").ap()
    out_ap = nc.dram_tensor("out", [C, H, W], BF16, kind="ExternalOutput").ap()
    with tile.TileContext(nc) as tc:
        _emit(tc, nc, x_ap, dk_ap, pm_dram, out_ap)
    nc.compile()
    _CACHED_NC = nc
    return nc


def pmask_np() -> np.ndarray:
    p = np.arange(128)
    return np.stack([(p % 2 == 0), (p % 2 == 1)], axis=1).astype(np.float32)


def make_in_maps(x: np.ndarray, dynamic_kernel: np.ndarray, n: int = NCORES):
    pm = pmask_np()
    return [
        {
            "x": np.ascontiguousarray(x[i], dtype=np.float32),
            "dynamic_kernel": np.ascontiguousarray(dynamic_kernel[i], dtype=np.float32),
            "pmask": pm,
        }
        for i in range(n)
    ]


def kernel(x: np.ndarray, dynamic_kernel: np.ndarray) -> np.ndarray:
    x = np.asarray(x)
    dynamic_kernel = np.asarray(dynamic_kernel)
    nc = build_nc()
    in_maps = make_in_maps(x, dynamic_kernel)
    res = run_bass_kernel_spmd(nc, in_maps, core_ids=list(range(NCORES)))
    out = np.stack([res.results[i]["out"] for i in range(NCORES)], axis=0)
    return out.astype(np.float32)


# revision 20
# speedup vs baseline: 1.0246x; 1.0246x over previous
"""AdaConv2d (per-pixel 3x3 dynamic conv) on 8 TRN2 NeuronCores.

out[b,c,h,w] = sum_t x_pad[b,c,h+dh(t),w+dw(t)] * dk[b,c,t,h,w]

Sharding: pure data parallel over batch (B=8 -> one batch element per core).

Per-core layout: partition p = 2c+s (c = channel 0..63, s = H-half 0..1); each
partition holds its half-plane of x as a 66-row bf16 SBUF tile (row 0 / row 65
are parity-masked halo rows, so every tap's product is one uniform full-height
DVE mul). dynamic_kernel streams through on the sync-engine DMA ring in
(row-block, tap) f32 tiles; the ACT engine converts each tile to bf16
(~3.4us/tile, ~65us total - under the ~102us DMA stream) so the DVE tap
products run in all-bf16 2x perf mode (~2.2us per 32-row tap, ~40us total).
DVE therefore can never pace the endgame: the kernel end is stream-end plus a
short convert->mul->matmul->copy->store chain on the final 8-row slice.
Row blocks: rows 0..32 as nine 32-row taps (16KB/lane packets, the DMA sweet
spot), rows 32..64 as a merged group accumulated in 16/8/8-row sub-blocks
whose closing taps are loaded as row-slices so each sub-block drains while
later slices still stream. W-boundary tap columns are memset (on gpsimd,
whose queue carries nothing else) in the bf16 product tile; the 9-tap sum
accumulates on the TensorEngine via identity-matmul into PSUM; ACT drains
PSUM to bf16 and issues the stores. Memory-bound: ~44 MB/core HBM traffic at
~400 GB/s/core effective.
"""

import numpy as np

from concourse import bacc, bass, tile
from concourse import mybir
from concourse.ap import AP
from concourse.bass_utils import run_bass_kernel_spmd
from concourse.masks import make_identity

F32 = mybir.dt.float32
BF16 = mybir.dt.bfloat16

B, C, H, W = 8, 64, 128, 128
K = 3
NTAP = K * K
NCORES = 8

HALF = H // 2           # 64 rows per half-plane
XROWS = HALF + 2        # 66 padded rows per partition

_CACHED_NC = None


def _emit(tc, nc, x_ap, dk_ap, pm_dram, out_ap):
    ctx_pools = []

    def pool(name, bufs, space=bass.MemorySpace.SBUF):
        p = tc.tile_pool(name=name, bufs=bufs, space=space)
        ctx_pools.append(p)
        return p.__enter__()

    try:
        const_pool = pool("const", 1)
        xbf_pool = pool("xbf", 1)
        dk_pool = pool("dk", 6)
        tmp_pool = pool("tmp", 7)
        out_pool = pool("osb", 3)
        psum_pool = pool("ps", 8, space=bass.MemorySpace.PSUM)

        identity = const_pool.tile([128, 128], BF16, name="identity")
        make_identity(nc, identity)

        # Partition p = 2c+s (c = channel, s = H-half). x_bf row 1+f holds
        # x[c, 64s+f]; row 0 is the top halo (x row 63 of the other half,
        # nonzero only for odd p) and row 65 the bottom halo (x row 64,
        # nonzero only for even p): every tap's product is ONE uniform
        # full-height DVE mul. W-boundary tap columns are memset in tmp.
        pm_ap = const_pool.tile([128, 2], F32, name="pm_ap")
        nc.scalar.dma_start(out=pm_ap[:], in_=pm_dram)
        halo_raw = const_pool.tile([128, 2, W], F32, name="halo_raw")
        nc.scalar.dma_start(
            out=halo_raw[:, 0:1, :],
            in_=AP(x_ap.tensor, HALF * W, [[H * W, C], [W, 2], [1, W]]),
        )
        nc.scalar.dma_start(
            out=halo_raw[:, 1:2, :],
            in_=AP(x_ap.tensor, (HALF - 2) * W, [[H * W, C], [W, 2], [1, W]]),
        )
        x_bf = xbf_pool.tile([128, XROWS, W], BF16, name="x_bf")
        nc.vector.tensor_scalar_mul(
            x_bf[:, 0:1, :], halo_raw[:, 1:2, :], pm_ap[:, 1:2]
        )
        nc.vector.tensor_scalar_mul(
            x_bf[:, HALF + 1 : HALF + 2, :], halo_raw[:, 0:1, :], pm_ap[:, 0:1]
        )

        def load_x_stage(lo, hi):
            # f32 staging load on the sync ring, ACT converts into x_bf
            xs = dk_pool.tile([128, hi - lo, W], F32, name=f"xs{lo}", tag="dk")
            nc.sync.dma_start(
                out=xs[:],
                in_=AP(
                    x_ap.tensor,
                    lo * W,
                    [[H * W, C], [HALF * W, 2], [W, hi - lo], [1, W]],
                ),
            )
            nc.scalar.copy(out=x_bf[:, lo + 1 : hi + 1, :], in_=xs[:])

        def load_dk(t, rlo, rhi, conv_gpsimd=False):
            # f32 tile streams in on the sync ring; ACT converts it straight
            # into the bf16 product tile (tmp), which the DVE then multiplies
            # by x IN PLACE (all-bf16 operands -> DVE 2x perf mode). The very
            # last slice converts on the idle GpSimd so its chain never waits
            # on ACT's convert backlog.
            dk_t = dk_pool.tile([128, rhi - rlo, W], F32, name="dk_t", tag="dk")
            nc.sync.dma_start(
                out=dk_t[:],
                in_=AP(
                    dk_ap.tensor,
                    t * H * W + rlo * W,
                    [[NTAP * H * W, C], [HALF * W, 2], [W, rhi - rlo], [1, W]],
                ),
            )
            tmp = tmp_pool.tile([128, rhi - rlo, W], BF16, name="tmp", tag="tmp")
            if conv_gpsimd:
                nc.gpsimd.tensor_copy(tmp[:], dk_t[:])
            else:
                nc.scalar.copy(out=tmp[:], in_=dk_t[:])
            return tmp

        def col_slices(dw):
            # valid output columns for this tap; the one boundary column
            # (reading x col -1 or W) contributes zero
            wo = slice(1, W) if dw < 0 else slice(0, W - 1) if dw > 0 else slice(0, W)
            return wo, slice(wo.start + dw, wo.stop + dw)

        # ---- block 0: output rows 0..32, nine monolithic 32-row taps
        # (16KB/lane packets). Its drain happens mid-stream with slack.
        load_x_stage(0, 32)
        load_x_stage(32, HALF)
        ps0 = [
            psum_pool.tile([128, 4, 128], F32, name=f"ps0_{j}", tag="ps")
            for j in range(8)
        ]
        order0 = [3, 4, 5, 6, 7, 0, 1, 2]
        for ti, t in enumerate(order0):
            dh, dw = t // K - 1, t % K - 1
            tmp = load_dk(t, 0, 32)
            wo, wx = col_slices(dw)
            if dw != 0:
                zc = slice(0, 1) if dw < 0 else slice(W - 1, W)
                nc.gpsimd.memset(tmp[:, :, zc], 0.0)
            nc.vector.tensor_mul(
                tmp[:, :, wo], x_bf[:, dh + 1 : dh + 33, wx], tmp[:, :, wo]
            )
            for j in range(8):
                nc.tensor.matmul(
                    ps0[j][:],
                    identity[:],
                    tmp[:, 4 * j : 4 * j + 4, :],
                    start=(ti == 0),
                    stop=False,
                )
        # closing tap in two 16-row halves: tiles 0-3 stop (and drain via
        # GpSimd) earlier, so the group's psum-slot handoff stalls less
        out_sb0 = out_pool.tile([128, 32, W], BF16, name="out_sb0", tag="osb")
        for h in range(2):
            dh, dw = 8 // K - 1, 8 % K - 1
            tmp = load_dk(8, 16 * h, 16 * h + 16)
            wo, wx = col_slices(dw)
            zc = slice(W - 1, W)
            nc.gpsimd.memset(tmp[:, :, zc], 0.0)
            nc.vector.tensor_mul(
                tmp[:, :, wo],
                x_bf[:, 16 * h + dh + 1 : 16 * h + dh + 17, wx],
                tmp[:, :, wo],
            )
            for j in range(4):
                nc.tensor.matmul(
                    ps0[4 * h + j][:],
                    identity[:],
                    tmp[:, 4 * j : 4 * j + 4, :],
                    start=False,
                    stop=True,
                )
            # drain this half's psum tiles immediately so the group's
            # matmuls can recycle them as early as possible
            for j in range(4):
                nc.vector.tensor_copy(
                    out_sb0[:, 16 * h + 4 * j : 16 * h + 4 * j + 4, :],
                    ps0[4 * h + j][:],
                )
        nc.scalar.dma_start(
            out=AP(out_ap.tensor, 0, [[H * W, C], [HALF * W, 2], [W, 32], [1, W]]),
            in_=out_sb0[:],
        )

        # ---- group: output rows 32..64 as three sub-blocks (16/8/8 rows)
        # sharing mostly-32-row dk loads (16KB packets); the closing tap
        # (t=4, center: no memset) is loaded in three row-slices so the
        # post-stream drain is a single 8-row mul -> 2 matmuls -> copies
        # -> store.
        G = 32
        sbs = []  # (group-row lo, hi, psum tiles)
        for lo, hi in [(0, 16), (16, 28), (28, 32)]:
            tiles = [
                psum_pool.tile([128, 4, 128], F32, name=f"psg_{lo}_{j}", tag="ps")
                for j in range((hi - lo) // 4)
            ]
            sbs.append((lo, hi, tiles))
        deferred_stores = []

        def group_step(t, glo, ghi, tmp):
            dh, dw = t // K - 1, t % K - 1
            wo, wx = col_slices(dw)
            # ONE fused mul per loaded slice (the sub-block split only
            # matters for psum/matmul grouping)
            if dw != 0:
                zc = slice(0, 1) if dw < 0 else slice(W - 1, W)
                nc.gpsimd.memset(tmp[:, :, zc], 0.0)
            nc.vector.tensor_mul(
                tmp[:, :, wo],
                x_bf[:, G + glo + dh + 1 : G + ghi + dh + 1, wx],
                tmp[:, :, wo],
            )
            for lo, hi, tiles in sbs:
                a, b_ = max(lo, glo), min(hi, ghi)
                if a >= b_:
                    continue
                for j in range(len(tiles)):
                    tr = lo + 4 * j  # group row of this psum tile
                    if tr < a or tr >= b_:
                        continue
                    nc.tensor.matmul(
                        tiles[j][:],
                        identity[:],
                        tmp[:, tr - glo : tr - glo + 4, :],
                        start=(t == 5),
                        stop=(t == 4),
                    )
                if t == 4:
                    # this sub-block just closed: drain (B1/B2 on ACT, the
                    # final 4-row B3 on Vector, which is idle after its last
                    # mul) + store. B1/B2 store dispatch rides the sync ring
                    # (emitted after the final dk load, so it cannot block
                    # the stream).
                    out_sb = out_pool.tile(
                        [128, hi - lo, W], BF16, name=f"out_g{lo}", tag="osb"
                    )
                    for j in range(len(tiles)):
                        # B3 (final 4 rows) drains on Vector, idle by then;
                        # B1/B2 drains fill ACT idle slots (the ready-first
                        # scheduler never lets them block a convert)
                        if lo == 28:
                            nc.vector.tensor_copy(
                                out_sb[:, 4 * j : 4 * j + 4, :], tiles[j][:]
                            )
                        else:
                            nc.scalar.copy(
                                out=out_sb[:, 4 * j : 4 * j + 4, :], in_=tiles[j][:]
                            )
                    out_dst = AP(
                        out_ap.tensor,
                        (G + lo) * W,
                        [[H * W, C], [HALF * W, 2], [W, hi - lo], [1, W]],
                    )
                    if lo in (0, 16):
                        deferred_stores.append((out_dst, out_sb))
                    else:
                        nc.scalar.dma_start(out=out_dst, in_=out_sb[:])

        # non-closing t3 slices ride mid-stream, and the last full tap (t2)
        # is split into 16-row halves, so every conv->mul->matmul chain for
        # a non-closing slice completes by about the time the stream ends;
        # the post-stream tail owes only the three closing t4 slices.
        seq_main = [(5, 0, 32), (6, 0, 32), (7, 0, 32), (8, 0, 32), (0, 0, 32),
                    (3, 0, 16), (1, 0, 32), (3, 16, 32), (2, 0, 16), (2, 16, 32)]
        for t, glo, ghi in seq_main:
            group_step(t, glo, ghi, load_dk(t, G + glo, G + ghi))
        # tail: emit ALL remaining loads + converts first (the final 4-row
        # slice converts on GpSimd, hitting zero convert backlog)
        tail = [(4, 0, 16), (4, 16, 28), (4, 28, 32)]
        tail_tmps = [
            load_dk(t, G + glo, G + ghi, conv_gpsimd=(glo == 28))
            for t, glo, ghi in tail
        ]
        for (t, glo, ghi), tmp in zip(tail, tail_tmps):
            group_step(t, glo, ghi, tmp)
        for dst, sb in deferred_stores:
            nc.sync.dma_start(out=dst, in_=sb[:])
    finally:
        for p in reversed(ctx_pools):
            p.__exit__(None, None, None)


def build_nc():
    global _CACHED_NC
    if _CACHED_NC is not None:
        return _CACHED_NC
    nc = bacc.Bacc("TRN2", target_bir_lowering=False, debug=False, num_devices=NCORES)
    x_ap = nc.dram_tensor("x", [C, H, W], F32, kind="ExternalInput").ap()
    dk_ap = nc.dram_tensor(
        "dynamic_kernel", [C, NTAP, H, W], F32, kind="ExternalInput"
    ).ap()
    pm_dram = nc.dram_tensor("pmask", [128, 2], F32, kind="ExternalInput").ap()
    out_ap = nc.dram_tensor("out", [C, H, W], BF16, kind="ExternalOutput").ap()
    with tile.TileContext(nc) as tc:
        _emit(tc, nc, x_ap, dk_ap, pm_dram, out_ap)
    nc.compile()
    _CACHED_NC = nc
    return nc


def pmask_np() -> np.ndarray:
    p = np.arange(128)
    return np.stack([(p % 2 == 0), (p % 2 == 1)], axis=1).astype(np.float32)


def make_in_maps(x: np.ndarray, dynamic_kernel: np.ndarray, n: int = NCORES):
    pm = pmask_np()
    return [
        {
            "x": np.ascontiguousarray(x[i], dtype=np.float32),
            "dynamic_kernel": np.ascontiguousarray(dynamic_kernel[i], dtype=np.float32),
            "pmask": pm,
        }
        for i in range(n)
    ]


def kernel(x: np.ndarray, dynamic_kernel: np.ndarray) -> np.ndarray:
    x = np.asarray(x)
    dynamic_kernel = np.asarray(dynamic_kernel)
    nc = build_nc()
    in_maps = make_in_maps(x, dynamic_kernel)
    res = run_bass_kernel_spmd(nc, in_maps, core_ids=list(range(NCORES)))
    out = np.stack([res.results[i]["out"] for i in range(NCORES)], axis=0)
    return out.astype(np.float32)
